# revision 6
# baseline (speedup 1.0000x reference)
"""Trainium2 Bass kernel for the Neural-CDE-style cell (nn_JaCDE_88167088653055).

Math (per batch row b):
    x    = spline(coeffs, t)   xdot = spline(dcoeffs, t)
    l1   = x @ wx.T + h @ wh.T + b0
    relu = relu(l1);  drelu = sigmoid(l1)
    lout = relu @ wout.T + b1; th = tanh(lout); dth = 1 - th^2
    J(v) = dth * ((drelu * v) @ wout.T)        # action of the Jacobian factor
    jx   = J(xdot @ wx.T); jxh = J(jx @ wh.T); jxhh = J(jxh @ wh.T)
    out  = jx + jxh + jxhh

Device-side reformulation (all bf16 on the PE path; tolerance is 2e-2):
  * spline eval (4-term polynomial over host-gathered coeffs) runs host-side;
    x/xdot are [64, N] and stack on partitions 0:64 / 64:128 of one tile, so
    their two K=64 matmuls run in different PE row groups.
  * sign-flip trick: the xdot weight copy and the wh copy used by the
    Jacobian-chain matmuls are negated HOST-side, so every m_i arrives
    negated and dth*m_i == (th^2-1)*(-m_i) is a single STT per term:
    jx = (w - 1) * m~ with w = th*th computed on GpSimd.
  * engine balance per chunk: ACT = {sigmoid, relu, tanh};
    DVE = {p1, jx, p2, jxh, p3, jxhh} (PSUM-sourced products);
    GpSimd = {w = th^2, t12 = jx+jxh, out = t12+jxhh} (bf16 SBUF).
  * l1 and u share one PSUM bank ([:,0:C] / [:,C:2C]) freeing banks for a
    6-deep rotating chain pool -> 4 chunks pipeline without slot stalls.
  * one packed input DMA per chunk; the ACT table sets are preloaded at t=0
    via a dummy sigmoid so no table load lands mid-kernel.

Sharding: pure data parallel - batch 8192 split as 1024 rows per core across
8 cores; small weights replicated. Activations are feature-major
([feature<=128 partitions, batch free]); every matmul is out.T = W @ act.T
with the contraction on partitions.
"""

import ml_dtypes
import numpy as np

import concourse.bass as bass
import concourse.mybir as mybir
import concourse.tile as tile
from concourse import bacc, bass_utils

N_CORES = 8
B = 8192
NOBS = 16
CIN = 64
H = 128
BS = B // N_CORES       # 1024 batch rows per core
CHUNK = 256             # batch columns per pipeline step
NCH = BS // CHUNK
F32 = mybir.dt.float32
BF16 = mybir.dt.bfloat16
NPBF = ml_dtypes.bfloat16

# input pack (bf16, per chunk): [128, 2*CHUNK]
#   cols [0:C)    partitions 0:64 = x.T, partitions 64:128 = xdot.T
#   cols [C:2C)   h.T
PACKW = 2 * CHUNK

_NC_CACHE = {}


def _build_nc():
    AF = mybir.ActivationFunctionType
    OP = mybir.AluOpType

    nc = bacc.Bacc("TRN2", target_bir_lowering=False, debug=False,
                   enable_asserts=False, num_devices=N_CORES)

    inb = nc.dram_tensor("inb", [NCH, 128, PACKW], BF16, kind="ExternalInput")
    # [wxx2 | wh | -wh | wout] as lhsT blocks; wxx2 rows 0:64 = wx-fold for x,
    # rows 64:128 = NEGATED wx-fold for xdot.
    wpack = nc.dram_tensor("wpack", [128, 4 * H], BF16, kind="ExternalInput")
    bpack = nc.dram_tensor("bpack", [128, 2], F32, kind="ExternalInput")
    outt = nc.dram_tensor("outt", [H, BS], BF16, kind="ExternalOutput")

    def mm(out_ap, lhsT, rhs, start=True, stop=True):
        nc.tensor.matmul(out_ap, lhsT, rhs, start=start, stop=stop,
                         skip_group_check=True)

    with tile.TileContext(nc) as tc:
        with tc.tile_pool(name="w", bufs=1) as wp, \
             tc.tile_pool(name="io", bufs=3) as io, \
             tc.tile_pool(name="tmp", bufs=4) as tmp, \
             tc.tile_pool(name="ps", bufs=2, space="PSUM") as ps, \
             tc.tile_pool(name="psc", bufs=4, space="PSUM") as psc:

            # --- constants --------------------------------------------------
            ws = wp.tile([128, 4 * H], BF16, tag="ws")
            nc.scalar.dma_start(ws[:], wpack[:])
            bs_ = wp.tile([128, 2], F32, tag="bs")
            nc.scalar.dma_start(bs_[:], bpack[:])
            wxx = ws[:, 0:H]            # [128, 128]: top 64 rows x, bottom -xdot
            whs = ws[:, H:2 * H]        # +wh (for l1)
            whsn = ws[:, 2 * H:3 * H]   # -wh (for the Jacobian chain)
            wos = ws[:, 3 * H:4 * H]    # wout
            b0s = bs_[:, 0:1]
            b1s = bs_[:, 1:2]

            # dummy sigmoid: forces the ACT table-set load(s) at t=0.
            dum = wp.tile([128, 1], F32, tag="dum")
            nc.scalar.activation(dum[:], bs_[:, 0:1], AF.Sigmoid)

            for ch in range(NCH):
                cs = bass.ts(ch, CHUNK)

                it = io.tile([128, PACKW], BF16, tag="it")
                nc.sync.dma_start(it[:], inb[ch])
                xxd = it[:, 0:CHUNK]
                hts = it[:, CHUNK:2 * CHUNK]

                l1t = ps.tile([H, CHUNK], F32, tag="l1")
                ut = ps.tile([H, CHUNK], F32, tag="u")
                l1 = l1t[:]
                u = ut[:]
                mm(u, wxx[64:128, :], xxd[64:128, :], start=True, stop=True)
                mm(l1, wxx[0:64, :], xxd[0:64, :], start=True, stop=False)
                mm(l1, whs, hts, start=False, stop=True)

                dr = tmp.tile([H, CHUNK], BF16, tag="dr")
                nc.scalar.activation(dr[:], l1, AF.Sigmoid, bias=b0s)
                relu = tmp.tile([H, CHUNK], BF16, tag="relu")
                nc.scalar.activation(relu[:], l1, AF.Relu, bias=b0s)

                lout = psc.tile([H, CHUNK], F32, tag="chain")
                mm(lout[:], wos, relu[:])
                th = tmp.tile([H, CHUNK], BF16, tag="th")
                nc.scalar.activation(th[:], lout[:], AF.Tanh, bias=b1s)
                w = tmp.tile([H, CHUNK], BF16, tag="w")
                nc.gpsimd.tensor_mul(w[:], th[:], th[:])

                # p1 = drelu * (-u)
                p1 = tmp.tile([H, CHUNK], BF16, tag="p1")
                nc.vector.tensor_mul(p1[:], dr[:], u)
                m1 = psc.tile([H, CHUNK], F32, tag="chain")
                mm(m1[:], wos, p1[:])                      # = -m1

                # jx = dth*m1 = (th^2-1)*(-m1)
                jx = tmp.tile([H, CHUNK], BF16, tag="jx")
                nc.vector.scalar_tensor_tensor(jx[:], w[:], 1.0, m1[:],
                                               OP.subtract, OP.mult)
                g1 = psc.tile([H, CHUNK], F32, tag="chain")
                mm(g1[:], whsn, jx[:])                     # = -g1
                p2 = tmp.tile([H, CHUNK], BF16, tag="p2")
                nc.vector.tensor_mul(p2[:], dr[:], g1[:])  # = -p2
                m2 = psc.tile([H, CHUNK], F32, tag="chain")
                mm(m2[:], wos, p2[:])                      # = -m2

                jxh = tmp.tile([H, CHUNK], BF16, tag="jxh")
                nc.vector.scalar_tensor_tensor(jxh[:], w[:], 1.0, m2[:],
                                               OP.subtract, OP.mult)
                t12 = tmp.tile([H, CHUNK], BF16, tag="t12")
                nc.gpsimd.tensor_add(t12[:], jx[:], jxh[:])

                g2 = psc.tile([H, CHUNK], F32, tag="chain")
                mm(g2[:], whsn, jxh[:])                    # = -g2
                p3 = tmp.tile([H, CHUNK], BF16, tag="p3")
                nc.vector.tensor_mul(p3[:], dr[:], g2[:])  # = -p3
                m3 = psc.tile([H, CHUNK], F32, tag="chain")
                mm(m3[:], wos, p3[:])                      # = -m3

                jxhh = tmp.tile([H, CHUNK], BF16, tag="jxhh")
                nc.vector.scalar_tensor_tensor(jxhh[:], w[:], 1.0, m3[:],
                                               OP.subtract, OP.mult)
                outs = tmp.tile([H, CHUNK], BF16, tag="outs")
                nc.gpsimd.tensor_add(outs[:], t12[:], jxhh[:])
                nc.sync.dma_start(outt[:, cs], outs[:])

    nc.compile()
    return nc


def _get_nc():
    if "nc" not in _NC_CACHE:
        _NC_CACHE["nc"] = _build_nc()
    return _NC_CACHE["nc"]


def _prep_in_maps(t, h, coeffs, dcoeffs, tobs, wx, wh, wout, b0, b1):
    t = np.asarray(t, np.float32)
    h = np.asarray(h, np.float32)
    coeffs = np.asarray(coeffs, np.float32)
    dcoeffs = np.asarray(dcoeffs, np.float32)
    tobs = np.asarray(tobs, np.float32)
    wx = np.asarray(wx, np.float32)
    wh = np.asarray(wh, np.float32)
    wout = np.asarray(wout, np.float32)
    b0 = np.asarray(b0, np.float32)
    b1 = np.asarray(b1, np.float32)

    ts = t[0]
    idx = int(np.clip(np.searchsorted(tobs, ts, side="right") - 1, 0, NOBS - 2))
    dtv = np.float32(ts - tobs[idx])
    powers = dtv ** np.arange(4, dtype=np.float32)            # [4]

    # host-side spline eval: x[b,c] = sum_j coeffs[b,idx,c,j] * dt^j
    x = coeffs[:, idx] @ powers                               # [B, CIN]
    xdot = dcoeffs[:, idx] @ powers                           # [B, CIN]

    # weights pack [128, 512] bf16: [wxx2 | wh.T | -wh.T | wout.T]
    wxx2 = np.concatenate([wx.T, -wx.T], axis=0)              # [128, 128]
    wpack = np.concatenate([wxx2, wh.T, -wh.T, wout.T],
                           axis=1).astype(NPBF)
    bpack = np.stack([b0, b1], axis=1).astype(np.float32)     # [128, 2]

    xb = x.astype(NPBF)
    xdb = xdot.astype(NPBF)
    hb = h.astype(NPBF)

    in_maps = []
    for c in range(N_CORES):
        sl = slice(c * BS, (c + 1) * BS)
        xt = xb[sl].T                                         # [64, BS]
        xdt = xdb[sl].T
        ht = hb[sl].T                                         # [128, BS]
        inb = np.empty((NCH, 128, PACKW), NPBF)
        for ch in range(NCH):
            cls = slice(ch * CHUNK, (ch + 1) * CHUNK)
            inb[ch, 0:64, 0:CHUNK] = xt[:, cls]
            inb[ch, 64:128, 0:CHUNK] = xdt[:, cls]
            inb[ch, :, CHUNK:2 * CHUNK] = ht[:, cls]
        in_maps.append({"inb": inb, "wpack": wpack, "bpack": bpack})
    return in_maps


def kernel(**inputs) -> np.ndarray:
    in_maps = _prep_in_maps(**inputs)
    nc = _get_nc()
    res = bass_utils.run_bass_kernel_spmd(nc, in_maps,
                                          core_ids=list(range(N_CORES)))
    out = np.empty((B, H), np.float32)
    for c in range(N_CORES):
        out[c * BS:(c + 1) * BS] = res.results[c]["outt"].T.astype(np.float32)
    return out


# revision 9
# speedup vs baseline: 1.1433x; 1.1433x over previous
"""Trainium2 Bass kernel for the Neural-CDE-style cell (nn_JaCDE_88167088653055).

Math (per batch row b):
    x    = spline(coeffs, t)   xdot = spline(dcoeffs, t)
    l1   = x @ wx.T + h @ wh.T + b0
    relu = relu(l1);  drelu = sigmoid(l1)
    lout = relu @ wout.T + b1; th = tanh(lout); dth = 1 - th^2
    J(v) = dth * ((drelu * v) @ wout.T)        # action of the Jacobian factor
    jx   = J(xdot @ wx.T); jxh = J(jx @ wh.T); jxhh = J(jxh @ wh.T)
    out  = jx + jxh + jxhh

Device-side reformulation (all bf16 on the PE path; tolerance is 2e-2):
  * spline eval (4-term polynomial over host-gathered coeffs) runs host-side;
    x/xdot are [64, N] and stack on partitions 0:64 / 64:128 of one tile, so
    their two K=64 matmuls run in different PE row groups.
  * sign-flip trick: the xdot weight copy and the wh copy used by the
    Jacobian-chain matmuls are negated HOST-side, so every m_i arrives
    negated and dth*m_i == (th^2-1)*(-m_i) is a single STT per term:
    jx = (w - 1) * m~ with w = th*th computed on GpSimd.
  * engine balance per chunk: ACT = {sigmoid, relu, tanh};
    DVE = {p1, jx, p2, jxh, p3, jxhh} (PSUM-sourced products);
    GpSimd = {w = th^2, t12 = jx+jxh, out = t12+jxhh} (bf16 SBUF).
  * l1 and u share one PSUM bank ([:,0:C] / [:,C:2C]) freeing banks for a
    6-deep rotating chain pool -> 4 chunks pipeline without slot stalls.
  * one packed input DMA per chunk; the ACT table sets are preloaded at t=0
    via a dummy sigmoid so no table load lands mid-kernel.

Sharding: pure data parallel - batch 8192 split as 1024 rows per core across
8 cores; small weights replicated. Activations are feature-major
([feature<=128 partitions, batch free]); every matmul is out.T = W @ act.T
with the contraction on partitions.
"""

import ml_dtypes
import numpy as np

import concourse.bass as bass
import concourse.mybir as mybir
import concourse.tile as tile
from concourse import bacc, bass_utils

N_CORES = 8
B = 8192
NOBS = 16
CIN = 64
H = 128
BS = B // N_CORES       # 1024 batch rows per core
CHUNK = 256             # batch columns per pipeline step
NCH = BS // CHUNK
F32 = mybir.dt.float32
BF16 = mybir.dt.bfloat16
NPBF = ml_dtypes.bfloat16

# input pack (bf16, per chunk): [128, 2*CHUNK]
#   cols [0:C)    partitions 0:64 = x.T, partitions 64:128 = xdot.T
#   cols [C:2C)   h.T
PACKW = 2 * CHUNK

_NC_CACHE = {}


def _build_nc():
    AF = mybir.ActivationFunctionType
    OP = mybir.AluOpType

    nc = bacc.Bacc("TRN2", target_bir_lowering=False, debug=False,
                   enable_asserts=False, num_devices=N_CORES)

    inb = nc.dram_tensor("inb", [NCH, 128, PACKW], BF16, kind="ExternalInput")
    # [wxx2 | wh | -wh | wout] as lhsT blocks; wxx2 rows 0:64 = wx-fold for x,
    # rows 64:128 = NEGATED wx-fold for xdot.
    wpack = nc.dram_tensor("wpack", [128, 4 * H], BF16, kind="ExternalInput")
    bpack = nc.dram_tensor("bpack", [128, 2], F32, kind="ExternalInput")
    outt = nc.dram_tensor("outt", [H, BS], BF16, kind="ExternalOutput")

    def mm(out_ap, lhsT, rhs, start=True, stop=True):
        nc.tensor.matmul(out_ap, lhsT, rhs, start=start, stop=stop,
                         skip_group_check=True)

    with tile.TileContext(nc) as tc:
        with tc.tile_pool(name="w", bufs=1) as wp, \
             tc.tile_pool(name="io", bufs=3) as io, \
             tc.tile_pool(name="tmp", bufs=4) as tmp, \
             tc.tile_pool(name="ps", bufs=2, space="PSUM") as ps, \
             tc.tile_pool(name="psc", bufs=4, space="PSUM") as psc:

            # --- constants --------------------------------------------------
            ws = wp.tile([128, 4 * H], BF16, tag="ws")
            nc.scalar.dma_start(ws[:], wpack[:])
            bs_ = wp.tile([128, 2], F32, tag="bs")
            nc.scalar.dma_start(bs_[:], bpack[:])
            wxx = ws[:, 0:H]            # [128, 128]: top 64 rows x, bottom -xdot
            whs = ws[:, H:2 * H]        # +wh (for l1)
            whsn = ws[:, 2 * H:3 * H]   # -wh (for the Jacobian chain)
            wos = ws[:, 3 * H:4 * H]    # wout
            b0s = bs_[:, 0:1]
            b1s = bs_[:, 1:2]

            # dummy sigmoid: forces the ACT table-set load(s) at t=0.
            dum = wp.tile([128, 1], F32, tag="dum")
            nc.scalar.activation(dum[:], bs_[:, 0:1], AF.Sigmoid)

            # PE warm-up: the HAM clock gate keeps the PE at 1.2 GHz until it
            # sees ~3.4us of sustained matmul activity, and this kernel's real
            # matmuls are too sparse to ever trip it (every MM measured at the
            # cold rate). Burn ~4us of dummy matmuls on garbage SBUF data
            # during the input-DMA wait (PE is idle until ~10.5us anyway) so
            # the real work runs at 2.4 GHz.
            wdum = wp.tile([128, 512], BF16, tag="wdum")
            nc.vector.memset(wdum[:], 0.0)
            pdum = psc.tile([H, 512], F32, tag="chain")
            for _ in range(10):
                mm(pdum[:], wdum[:, 0:128], wdum[:], start=True, stop=True)

            for ch in range(NCH):
                cs = bass.ts(ch, CHUNK)

                it = io.tile([128, PACKW], BF16, tag="it")
                nc.sync.dma_start(it[:], inb[ch])
                xxd = it[:, 0:CHUNK]
                hts = it[:, CHUNK:2 * CHUNK]

                l1t = ps.tile([H, CHUNK], F32, tag="l1")
                ut = ps.tile([H, CHUNK], F32, tag="u")
                l1 = l1t[:]
                u = ut[:]
                mm(u, wxx[64:128, :], xxd[64:128, :], start=True, stop=True)
                mm(l1, wxx[0:64, :], xxd[0:64, :], start=True, stop=False)
                mm(l1, whs, hts, start=False, stop=True)

                dr = tmp.tile([H, CHUNK], BF16, tag="dr")
                nc.scalar.activation(dr[:], l1, AF.Sigmoid, bias=b0s)
                relu = tmp.tile([H, CHUNK], BF16, tag="relu")
                nc.scalar.activation(relu[:], l1, AF.Relu, bias=b0s)

                lout = psc.tile([H, CHUNK], F32, tag="chain")
                mm(lout[:], wos, relu[:])
                th = tmp.tile([H, CHUNK], BF16, tag="th")
                nc.scalar.activation(th[:], lout[:], AF.Tanh, bias=b1s)
                w = tmp.tile([H, CHUNK], BF16, tag="w")
                nc.gpsimd.tensor_mul(w[:], th[:], th[:])

                # p1 = drelu * (-u)
                p1 = tmp.tile([H, CHUNK], BF16, tag="p1")
                nc.vector.tensor_mul(p1[:], dr[:], u)
                m1 = psc.tile([H, CHUNK], F32, tag="chain")
                mm(m1[:], wos, p1[:])                      # = -m1

                # jx = dth*m1 = (th^2-1)*(-m1)
                jx = tmp.tile([H, CHUNK], BF16, tag="jx")
                nc.vector.scalar_tensor_tensor(jx[:], w[:], 1.0, m1[:],
                                               OP.subtract, OP.mult)
                g1 = psc.tile([H, CHUNK], F32, tag="chain")
                mm(g1[:], whsn, jx[:])                     # = -g1
                p2 = tmp.tile([H, CHUNK], BF16, tag="p2")
                nc.vector.tensor_mul(p2[:], dr[:], g1[:])  # = -p2
                m2 = psc.tile([H, CHUNK], F32, tag="chain")
                mm(m2[:], wos, p2[:])                      # = -m2

                jxh = tmp.tile([H, CHUNK], BF16, tag="jxh")
                nc.vector.scalar_tensor_tensor(jxh[:], w[:], 1.0, m2[:],
                                               OP.subtract, OP.mult)
                t12 = tmp.tile([H, CHUNK], BF16, tag="t12")
                nc.gpsimd.tensor_add(t12[:], jx[:], jxh[:])

                g2 = psc.tile([H, CHUNK], F32, tag="chain")
                mm(g2[:], whsn, jxh[:])                    # = -g2
                p3 = tmp.tile([H, CHUNK], BF16, tag="p3")
                nc.vector.tensor_mul(p3[:], dr[:], g2[:])  # = -p3
                m3 = psc.tile([H, CHUNK], F32, tag="chain")
                mm(m3[:], wos, p3[:])                      # = -m3

                jxhh = tmp.tile([H, CHUNK], BF16, tag="jxhh")
                nc.vector.scalar_tensor_tensor(jxhh[:], w[:], 1.0, m3[:],
                                               OP.subtract, OP.mult)
                outs = tmp.tile([H, CHUNK], BF16, tag="outs")
                if ch == NCH - 1:
                    # last chunk's sum gates the kernel tail: DVE is ~2x
                    # faster than GpSimd for this bf16 SBUF add.
                    nc.vector.tensor_add(outs[:], t12[:], jxhh[:])
                else:
                    nc.gpsimd.tensor_add(outs[:], t12[:], jxhh[:])
                nc.sync.dma_start(outt[:, cs], outs[:])

    nc.compile()
    return nc


def _get_nc():
    if "nc" not in _NC_CACHE:
        _NC_CACHE["nc"] = _build_nc()
    return _NC_CACHE["nc"]


def _prep_in_maps(t, h, coeffs, dcoeffs, tobs, wx, wh, wout, b0, b1):
    t = np.asarray(t, np.float32)
    h = np.asarray(h, np.float32)
    coeffs = np.asarray(coeffs, np.float32)
    dcoeffs = np.asarray(dcoeffs, np.float32)
    tobs = np.asarray(tobs, np.float32)
    wx = np.asarray(wx, np.float32)
    wh = np.asarray(wh, np.float32)
    wout = np.asarray(wout, np.float32)
    b0 = np.asarray(b0, np.float32)
    b1 = np.asarray(b1, np.float32)

    ts = t[0]
    idx = int(np.clip(np.searchsorted(tobs, ts, side="right") - 1, 0, NOBS - 2))
    dtv = np.float32(ts - tobs[idx])
    powers = dtv ** np.arange(4, dtype=np.float32)            # [4]

    # host-side spline eval: x[b,c] = sum_j coeffs[b,idx,c,j] * dt^j
    x = coeffs[:, idx] @ powers                               # [B, CIN]
    xdot = dcoeffs[:, idx] @ powers                           # [B, CIN]

    # weights pack [128, 512] bf16: [wxx2 | wh.T | -wh.T | wout.T]
    wxx2 = np.concatenate([wx.T, -wx.T], axis=0)              # [128, 128]
    wpack = np.concatenate([wxx2, wh.T, -wh.T, wout.T],
                           axis=1).astype(NPBF)
    bpack = np.stack([b0, b1], axis=1).astype(np.float32)     # [128, 2]

    xb = x.astype(NPBF)
    xdb = xdot.astype(NPBF)
    hb = h.astype(NPBF)

    in_maps = []
    for c in range(N_CORES):
        sl = slice(c * BS, (c + 1) * BS)
        xt = xb[sl].T                                         # [64, BS]
        xdt = xdb[sl].T
        ht = hb[sl].T                                         # [128, BS]
        inb = np.empty((NCH, 128, PACKW), NPBF)
        for ch in range(NCH):
            cls = slice(ch * CHUNK, (ch + 1) * CHUNK)
            inb[ch, 0:64, 0:CHUNK] = xt[:, cls]
            inb[ch, 64:128, 0:CHUNK] = xdt[:, cls]
            inb[ch, :, CHUNK:2 * CHUNK] = ht[:, cls]
        in_maps.append({"inb": inb, "wpack": wpack, "bpack": bpack})
    return in_maps


def kernel(**inputs) -> np.ndarray:
    in_maps = _prep_in_maps(**inputs)
    nc = _get_nc()
    res = bass_utils.run_bass_kernel_spmd(nc, in_maps,
                                          core_ids=list(range(N_CORES)))
    out = np.empty((B, H), np.float32)
    for c in range(N_CORES):
        out[c * BS:(c + 1) * BS] = res.results[c]["outt"].T.astype(np.float32)
    return out


# revision 10
# speedup vs baseline: 1.5780x; 1.3802x over previous
"""Trainium2 Bass kernel for the Neural-CDE-style cell (nn_JaCDE_88167088653055).

Math (per batch row b):
    x    = spline(coeffs, t)   xdot = spline(dcoeffs, t)
    l1   = x @ wx.T + h @ wh.T + b0
    relu = relu(l1);  drelu = sigmoid(l1)
    lout = relu @ wout.T + b1; th = tanh(lout); dth = 1 - th^2
    J(v) = dth * ((drelu * v) @ wout.T)        # action of the Jacobian factor
    jx   = J(xdot @ wx.T); jxh = J(jx @ wh.T); jxhh = J(jxh @ wh.T)
    out  = jx + jxh + jxhh

Device-side reformulation (all bf16 on the PE path; tolerance is 2e-2):
  * spline eval (4-term polynomial over host-gathered coeffs) runs host-side;
    x/xdot are [64, N] and stack on partitions 0:64 / 64:128 of one tile, so
    their two K=64 matmuls run in different PE row groups.
  * sign-flip trick: the xdot weight copy and the wh copy used by the
    Jacobian-chain matmuls are negated HOST-side, so every m_i arrives
    negated and dth*m_i == (th^2-1)*(-m_i) is a single STT per term, with
    th^2 from ACT Square (tanh and square share one ACT table set).
  * chunks are processed in software-pipelined PAIRS: instructions are
    emitted stage-major across each pair, so every engine's strict-FIFO
    queue always holds the twin chunk's op to fill dependency stalls
    (chunk-major order serializes the whole Jacobian chain per chunk).
  * PE warm-up burst: ~4us of dummy matmuls during the input-DMA wait flip
    the HAM clock gate to 2.4 GHz before real work arrives (this kernel's
    matmuls are otherwise too sparse to ever leave the cold 1.2 GHz state).
  * engine balance per chunk: ACT = {relu, sigmoid, tanh, square};
    DVE = {p1, jx, p2, jxh, p3, jxhh}; GpSimd = {t12, outs}.

Sharding: pure data parallel - batch 8192 split as 1024 rows per core across
8 cores; small weights replicated. Activations are feature-major
([feature<=128 partitions, batch free]); every matmul is out.T = W @ act.T
with the contraction on partitions.
"""

import ml_dtypes
import numpy as np

import concourse.bass as bass
import concourse.mybir as mybir
import concourse.tile as tile
from concourse import bacc, bass_utils

N_CORES = 8
B = 8192
NOBS = 16
CIN = 64
H = 128
BS = B // N_CORES       # 1024 batch rows per core
CHUNK = 256             # batch columns per pipeline step
NCH = BS // CHUNK
F32 = mybir.dt.float32
BF16 = mybir.dt.bfloat16
NPBF = ml_dtypes.bfloat16

# input pack (bf16, per chunk): [128, 2*CHUNK]
#   cols [0:C)    partitions 0:64 = x.T, partitions 64:128 = xdot.T
#   cols [C:2C)   h.T
PACKW = 2 * CHUNK

_NC_CACHE = {}


def _build_nc():
    AF = mybir.ActivationFunctionType
    OP = mybir.AluOpType

    nc = bacc.Bacc("TRN2", target_bir_lowering=False, debug=False,
                   enable_asserts=False, num_devices=N_CORES)

    inb = nc.dram_tensor("inb", [NCH, 128, PACKW], BF16, kind="ExternalInput")
    # [wxx2 | wh | -wh | wout] as lhsT blocks; wxx2 rows 0:64 = wx-fold for x,
    # rows 64:128 = NEGATED wx-fold for xdot.
    wpack = nc.dram_tensor("wpack", [128, 4 * H], BF16, kind="ExternalInput")
    bpack = nc.dram_tensor("bpack", [128, 2], F32, kind="ExternalInput")
    outt = nc.dram_tensor("outt", [H, BS], BF16, kind="ExternalOutput")

    def mm(out_ap, lhsT, rhs, start=True, stop=True):
        nc.tensor.matmul(out_ap, lhsT, rhs, start=start, stop=stop,
                         skip_group_check=True)

    with tile.TileContext(nc) as tc:
        with tc.tile_pool(name="w", bufs=1) as wp, \
             tc.tile_pool(name="io", bufs=4) as io, \
             tc.tile_pool(name="tmp", bufs=4) as tmp, \
             tc.tile_pool(name="ps", bufs=2, space="PSUM") as ps, \
             tc.tile_pool(name="psc", bufs=4, space="PSUM") as psc:

            # --- constants --------------------------------------------------
            ws = wp.tile([128, 4 * H], BF16, tag="ws")
            nc.scalar.dma_start(ws[:], wpack[:])
            bs_ = wp.tile([128, 2], F32, tag="bs")
            nc.scalar.dma_start(bs_[:], bpack[:])
            wxx = ws[:, 0:H]            # [128, 128]: top 64 rows x, bottom -xdot
            whs = ws[:, H:2 * H]        # +wh (for l1)
            whsn = ws[:, 2 * H:3 * H]   # -wh (for the Jacobian chain)
            wos = ws[:, 3 * H:4 * H]    # wout
            b0s = bs_[:, 0:1]
            b1s = bs_[:, 1:2]

            # dummy sigmoid: forces the ACT table-set load(s) at t=0.
            dum = wp.tile([128, 1], F32, tag="dum")
            nc.scalar.activation(dum[:], bs_[:, 0:1], AF.Sigmoid)

            # PE warm-up (see module docstring). memset on GpSimd so the
            # burst can start as soon as the preamble ends.
            wdum = wp.tile([128, 512], BF16, tag="wdum")
            nc.gpsimd.memset(wdum[:], 0.0)
            pdum = psc.tile([H, 512], F32, tag="chain")
            for _ in range(9):
                mm(pdum[:], wdum[:, 0:128], wdum[:], start=True, stop=True)

            # all input DMAs issued up front on the Sync queue
            its = []
            for ch in range(NCH):
                it = io.tile([128, PACKW], BF16, tag="it")
                nc.sync.dma_start(it[:], inb[ch])
                its.append(it)

            T = {}  # (name, ch) -> tile

            def tt(name, ch, dtype=BF16):
                t = tmp.tile([H, CHUNK], dtype, tag=name)
                T[(name, ch)] = t
                return t

            for pair in range(NCH // 2):
                chs = (2 * pair, 2 * pair + 1)

                for ch in chs:
                    xxd = its[ch][:, 0:CHUNK]
                    hts = its[ch][:, CHUNK:2 * CHUNK]
                    u = ps.tile([H, CHUNK], F32, tag="u")
                    T[("u", ch)] = u
                    l1 = ps.tile([H, CHUNK], F32, tag="l1")
                    T[("l1", ch)] = l1
                    mm(u[:], wxx[64:128, :], xxd[64:128, :], start=True,
                       stop=True)
                    mm(l1[:], wxx[0:64, :], xxd[0:64, :], start=True,
                       stop=False)
                    mm(l1[:], whs, hts, start=False, stop=True)

                for ch in chs:
                    nc.scalar.activation(tt("relu", ch)[:], T[("l1", ch)][:],
                                         AF.Relu, bias=b0s)
                for ch in chs:
                    nc.scalar.activation(tt("dr", ch)[:], T[("l1", ch)][:],
                                         AF.Sigmoid, bias=b0s)
                for ch in chs:
                    lo = psc.tile([H, CHUNK], F32, tag="chain")
                    T[("lout", ch)] = lo
                    mm(lo[:], wos, T[("relu", ch)][:])
                for ch in chs:
                    nc.scalar.activation(tt("th", ch)[:], T[("lout", ch)][:],
                                         AF.Tanh, bias=b1s)
                for ch in chs:
                    nc.scalar.activation(tt("sq", ch)[:], T[("th", ch)][:],
                                         AF.Square)
                for ch in chs:
                    nc.vector.tensor_mul(tt("p1", ch)[:], T[("dr", ch)][:],
                                         T[("u", ch)][:])
                for ch in chs:
                    m1 = psc.tile([H, CHUNK], F32, tag="chain")
                    T[("m1", ch)] = m1
                    mm(m1[:], wos, T[("p1", ch)][:])          # = -m1
                for ch in chs:
                    # jx = dth*m1 = (th^2-1)*(-m1)
                    nc.vector.scalar_tensor_tensor(
                        tt("jx", ch)[:], T[("sq", ch)][:], 1.0,
                        T[("m1", ch)][:], OP.subtract, OP.mult)
                for ch in chs:
                    g1 = psc.tile([H, CHUNK], F32, tag="chain")
                    T[("g1", ch)] = g1
                    mm(g1[:], whsn, T[("jx", ch)][:])         # = -g1
                for ch in chs:
                    nc.vector.tensor_mul(tt("p2", ch)[:], T[("dr", ch)][:],
                                         T[("g1", ch)][:])    # = -p2
                for ch in chs:
                    m2 = psc.tile([H, CHUNK], F32, tag="chain")
                    T[("m2", ch)] = m2
                    mm(m2[:], wos, T[("p2", ch)][:])          # = -m2
                for ch in chs:
                    nc.vector.scalar_tensor_tensor(
                        tt("jxh", ch)[:], T[("sq", ch)][:], 1.0,
                        T[("m2", ch)][:], OP.subtract, OP.mult)
                for ch in chs:
                    nc.gpsimd.tensor_add(tt("t12", ch)[:], T[("jx", ch)][:],
                                         T[("jxh", ch)][:])
                for ch in chs:
                    g2 = psc.tile([H, CHUNK], F32, tag="chain")
                    T[("g2", ch)] = g2
                    mm(g2[:], whsn, T[("jxh", ch)][:])        # = -g2
                for ch in chs:
                    nc.vector.tensor_mul(tt("p3", ch)[:], T[("dr", ch)][:],
                                         T[("g2", ch)][:])    # = -p3
                for ch in chs:
                    m3 = psc.tile([H, CHUNK], F32, tag="chain")
                    T[("m3", ch)] = m3
                    mm(m3[:], wos, T[("p3", ch)][:])          # = -m3
                for ch in chs:
                    nc.vector.scalar_tensor_tensor(
                        tt("jxhh", ch)[:], T[("sq", ch)][:], 1.0,
                        T[("m3", ch)][:], OP.subtract, OP.mult)
                for ch in chs:
                    outs = tt("outs", ch)
                    if ch == NCH - 1:
                        # last chunk's sum gates the kernel tail: DVE is
                        # ~2x faster than GpSimd for this bf16 SBUF add.
                        nc.vector.tensor_add(outs[:], T[("t12", ch)][:],
                                             T[("jxhh", ch)][:])
                    else:
                        nc.gpsimd.tensor_add(outs[:], T[("t12", ch)][:],
                                             T[("jxhh", ch)][:])
                for ch in chs:
                    nc.sync.dma_start(outt[:, bass.ts(ch, CHUNK)],
                                      T[("outs", ch)][:])

    nc.compile()
    return nc


def _get_nc():
    if "nc" not in _NC_CACHE:
        _NC_CACHE["nc"] = _build_nc()
    return _NC_CACHE["nc"]


def _prep_in_maps(t, h, coeffs, dcoeffs, tobs, wx, wh, wout, b0, b1):
    t = np.asarray(t, np.float32)
    h = np.asarray(h, np.float32)
    coeffs = np.asarray(coeffs, np.float32)
    dcoeffs = np.asarray(dcoeffs, np.float32)
    tobs = np.asarray(tobs, np.float32)
    wx = np.asarray(wx, np.float32)
    wh = np.asarray(wh, np.float32)
    wout = np.asarray(wout, np.float32)
    b0 = np.asarray(b0, np.float32)
    b1 = np.asarray(b1, np.float32)

    ts = t[0]
    idx = int(np.clip(np.searchsorted(tobs, ts, side="right") - 1, 0, NOBS - 2))
    dtv = np.float32(ts - tobs[idx])
    powers = dtv ** np.arange(4, dtype=np.float32)            # [4]

    # host-side spline eval: x[b,c] = sum_j coeffs[b,idx,c,j] * dt^j
    x = coeffs[:, idx] @ powers                               # [B, CIN]
    xdot = dcoeffs[:, idx] @ powers                           # [B, CIN]

    # weights pack [128, 512] bf16: [wxx2 | wh.T | -wh.T | wout.T]
    wxx2 = np.concatenate([wx.T, -wx.T], axis=0)              # [128, 128]
    wpack = np.concatenate([wxx2, wh.T, -wh.T, wout.T],
                           axis=1).astype(NPBF)
    bpack = np.stack([b0, b1], axis=1).astype(np.float32)     # [128, 2]

    xb = x.astype(NPBF)
    xdb = xdot.astype(NPBF)
    hb = h.astype(NPBF)

    in_maps = []
    for c in range(N_CORES):
        sl = slice(c * BS, (c + 1) * BS)
        xt = xb[sl].T                                         # [64, BS]
        xdt = xdb[sl].T
        ht = hb[sl].T                                         # [128, BS]
        inb = np.empty((NCH, 128, PACKW), NPBF)
        for ch in range(NCH):
            cls = slice(ch * CHUNK, (ch + 1) * CHUNK)
            inb[ch, 0:64, 0:CHUNK] = xt[:, cls]
            inb[ch, 64:128, 0:CHUNK] = xdt[:, cls]
            inb[ch, :, CHUNK:2 * CHUNK] = ht[:, cls]
        in_maps.append({"inb": inb, "wpack": wpack, "bpack": bpack})
    return in_maps


def kernel(**inputs) -> np.ndarray:
    in_maps = _prep_in_maps(**inputs)
    nc = _get_nc()
    res = bass_utils.run_bass_kernel_spmd(nc, in_maps,
                                          core_ids=list(range(N_CORES)))
    out = np.empty((B, H), np.float32)
    for c in range(N_CORES):
        out[c * BS:(c + 1) * BS] = res.results[c]["outt"].T.astype(np.float32)
    return out
